# revision 13
# baseline (speedup 1.0000x reference)
"""Trainium2 Bass kernel for nn_CustomCIFAR10Model.

Math (reference):
    xf = x.reshape(B, D)
    part2[b,d] = cos(xf[b,d]) * Sa[d] + sin(xf[b,d]) * Sb[d]
        where Sa[d] = sum_i a[i,d,0], Sb[d] = sum_i b[i,d,0]
    part1 = sum(w[1:]*n[1:] + w[:-1]*n[:-1])            (scalar)
    out = (part1 + part2) @ fc_w.T + fc_b               [B, NCLS]

Memory-bound: the only heavy work is streaming a and b once to
column-sum them. Sharding: d-columns split across 8 cores (384 each);
every core reduces its a/b slice, scales its fc_w columns by the sums,
and contracts against cos/sin(x) to a partial [NCLS, B] output; the
host sums the 8 partials and adds part1/bias.

Schedule (evolved over several traced iterations):
 - a/b/x/fc_w cast to fp16 on the host: the stream halves to ~5.2 MB
   per core (adds ~4e-4 relative error, gate is 2e-2).
 - ALL load DMAs ride the single sync HWDGE ring, issued back-to-back
   at kernel start.  The stream is HBM-bound (~390-420 GB/s either
   way), and a single FIFO ring makes the arrival order deterministic:
   PE matmuls are emitted in exactly ring order, so the in-order PE
   never stalls on a not-yet-landed piece while a later piece sits
   ready (cost 2-4 us in every dual-ring variant).  It also keeps DMA
   issue off the ACT engine: a 9th+ DMA's issue blocks its engine
   until a completion-sem lane recycles, which on ACT pushed the Sin
   table load + all Sins back by ~4 us.
 - Piece order: combo (x+fc_w) first -> trig pipeline (DVE+ACT) runs
   11-16 us; a pieces small->big (PE starts ~10.7, stays fed); b big->
   small (the last piece is 2 chunks, so the post-stream tail starts
   with only ~0.3 us of reduction left).
 - Elementwise trig on DVE in fp16 (2x mode) + Sin on ACT.  The magic
   round constant stays 1.5*2^23: DVE computes INTERNALLY in fp32, so
   the fused (t+M)-M tensor_scalar rounds exactly and only the small
   integer result lands in fp16.  GpSimd runs nothing (its tensor ops
   measure ~20x slower than DVE, and its SWDGE DMA path took 10 us for
   a 0.15 MB tensor).
 - 14 dummy matmuls bridge the PE from preamble end to first data so
   the HAM clock-gate reaches 2.4 GHz (cold 1.2 GHz doubles the 7.7 us
   reduction; fewer warm-ups measurably failed to flip it).
 - Output store: PSUM->SBUF copy split by columns across ACT/DVE, then
   two partition-contiguous DMAs on the two HWDGE rings.

HW Sin only accepts [-pi, pi]: range-reduce t = x/(2pi), r = t - round(t)
via the magic trick, then Sin(2pi*r); cos shifts t by +1/4 first.
"""

import numpy as np

B = 512
D = 3072
NCLS = 100
P = 128
NCORES = 8
DW = D // NCORES          # 384 columns per core
NSUB = DW // P            # 3 d-subtiles of 128
NCH = D // P              # 24 row-chunks of the a/b slice

A_PIECES = [(0, 3), (3, 7), (7, 12), (12, 17), (17, 24)]
B_PIECES = [(0, 7), (7, 12), (12, 16), (16, 19), (19, 22), (22, 24)]

_STATE = {}


def _build():
    """Build + bacc-compile the SPMD Bass program (once per process)."""
    import concourse.bacc as bacc
    import concourse.mybir as mybir
    import concourse.tile as tile

    f32 = mybir.dt.float32
    f16 = mybir.dt.float16
    nc = bacc.Bacc(
        "TRN2", target_bir_lowering=False, debug=False, num_devices=NCORES
    )

    a_s = nc.dram_tensor("a_s", [P, NCH * DW], f16, kind="ExternalInput")
    b_s = nc.dram_tensor("b_s", [P, NCH * DW], f16, kind="ExternalInput")
    # combo: per partition p, per sub s: [ x (B) | fwt (NCLS) ]
    cmb_s = nc.dram_tensor("cmb_s", [P, NSUB * (B + NCLS)], f16, kind="ExternalInput")
    out_cb = nc.dram_tensor("out_cb", [NCLS, B], f32, kind="ExternalOutput")

    INV2PI = float(1.0 / (2.0 * np.pi))
    TWO_PI = float(2.0 * np.pi)
    MAGIC = float(1.5 * 2.0**23)
    add_op = mybir.AluOpType.add
    sub_op = mybir.AluOpType.subtract
    Sin = mybir.ActivationFunctionType.Sin

    with tile.TileContext(nc) as tc:
        with (
            tc.tile_pool(name="consts", bufs=1) as const_pool,
            tc.tile_pool(name="xwork", bufs=1) as x_pool,
            tc.tile_pool(name="ps", bufs=2, space="PSUM") as psum_pool,
            tc.tile_pool(name="psrow", bufs=1, space="PSUM") as psum_row_pool,
            tc.tile_pool(name="psout", bufs=1, space="PSUM") as psum_out_pool,
            tc.tile_pool(name="pswarm", bufs=1, space="PSUM") as psum_warm_pool,
        ):
            ones_h = const_pool.tile([P, 1], f16, name="ones_h")
            nc.vector.memset(ones_h[:], 1.0)
            one1 = const_pool.tile([1, 1], f32, name="one1")
            nc.vector.memset(one1[:], 1.0)
            zero = const_pool.tile([P, 1], f32, name="zerob")
            nc.vector.memset(zero[:], 0.0)
            zero_h = const_pool.tile([P, 1], f16, name="zero_h")
            nc.vector.memset(zero_h[:], 0.0)
            warmz = const_pool.tile([P, DW], f16, name="warmz")
            nc.vector.memset(warmz[:], 0.0)

            # ---------- load DMAs: one FIFO ring, deterministic order ----
            a_sb = x_pool.tile([P, NCH, DW], f16, name="a_sb")
            b_sb = x_pool.tile([P, NCH, DW], f16, name="b_sb")
            cmb = x_pool.tile([P, NSUB, B + NCLS], f16, name="cmb")

            nc.sync.dma_start(
                out=cmb[:], in_=cmb_s[:].rearrange("p (s c) -> p s c", s=NSUB)
            )
            for (c0, c1) in A_PIECES:
                nc.sync.dma_start(
                    out=a_sb[:, c0:c1, :], in_=a_s[:, c0 * DW : c1 * DW]
                )
            for (c0, c1) in B_PIECES:
                nc.sync.dma_start(
                    out=b_sb[:, c0:c1, :], in_=b_s[:, c0 * DW : c1 * DW]
                )

            # Dummy Sin so the Sin table set loads once at kernel start.
            warm = const_pool.tile([P, 1], f32, name="warm")
            nc.scalar.activation(warm[:], zero[:], Sin, bias=zero[:])

            # PE warm-up bridge: preamble end -> first data arrival.
            warm_ps = psum_warm_pool.tile([1, DW], f32, name="warm_ps")
            for _ in range(14):
                nc.tensor.matmul(
                    warm_ps[:], ones_h[:], warmz[:], start=True, stop=True
                )

            # ---------- column-sum reduction (PE) ----------
            rows = []
            for ti in range(2):
                psr = psum_row_pool.tile([1, DW], f32, name=f"psr{ti}", tag=f"psr{ti}")
                rows.append(psr)
            emitted = [0, 0]

            def chunk_mms(ti, src, c0, c1):
                for c in range(c0, c1):
                    nc.tensor.matmul(
                        rows[ti][:],
                        ones_h[:],
                        src[:, c, :],
                        start=(emitted[ti] == 0),
                        stop=(emitted[ti] == NCH - 1),
                    )
                    emitted[ti] += 1

            for (c0, c1) in A_PIECES:
                chunk_mms(0, a_sb, c0, c1)

            # ---------- trig on x: fp16 on DVE + ACT, cos side first ----
            xt = cmb[:, :, 0:B]
            fwt = cmb[:, :, B : B + NCLS]
            sins = []
            coss = []
            ts_ts = []
            for sub in range(NSUB):
                xts = xt[:, sub, :]
                ts_t = x_pool.tile([P, B], f16, name=f"ts{sub}")
                nc.vector.tensor_scalar_mul(ts_t[:], xts, INV2PI)
                ts_ts.append(ts_t)
                tc_t = x_pool.tile([P, B], f16, name=f"tc{sub}")
                nc.vector.tensor_scalar_add(tc_t[:], ts_t[:], 0.25)
                kc_t = x_pool.tile([P, B], f16, name=f"kc{sub}")
                nc.vector.tensor_scalar(kc_t[:], tc_t[:], MAGIC, MAGIC, add_op, sub_op)
                rc_t = x_pool.tile([P, B], f16, name=f"rc{sub}")
                nc.vector.tensor_sub(rc_t[:], tc_t[:], kc_t[:])
                cosv = x_pool.tile([P, B], f16, name=f"cos{sub}")
                nc.scalar.activation(cosv[:], rc_t[:], Sin, bias=zero_h[:], scale=TWO_PI)
                coss.append(cosv)
            for sub in range(NSUB):
                ks_t = x_pool.tile([P, B], f16, name=f"ks{sub}")
                nc.vector.tensor_scalar(ks_t[:], ts_ts[sub][:], MAGIC, MAGIC, add_op, sub_op)
                rs_t = x_pool.tile([P, B], f16, name=f"rs{sub}")
                nc.vector.tensor_sub(rs_t[:], ts_ts[sub][:], ks_t[:])
                sinv = x_pool.tile([P, B], f16, name=f"sin{sub}")
                nc.scalar.activation(sinv[:], rs_t[:], Sin, bias=zero_h[:], scale=TWO_PI)
                sins.append(sinv)

            out_ps = psum_out_pool.tile([NCLS, B], f32, name="out_ps")
            H = B // 2

            def make_fws(ti, sub):
                """Transpose row ti/sub to per-partition cols (via PE),
                scale the SMALL fwt tile by it (fwt[d,c]*S[d])."""
                row_sb = const_pool.tile(
                    [1, P], f32, name=f"row{ti}_{sub}", tag=f"row{ti}_{sub}"
                )
                nc.scalar.copy(row_sb[:], rows[ti][0:1, sub * P : (sub + 1) * P])
                ps = psum_pool.tile([P, 1], f32, name=f"ps{ti}_{sub}", tag="ps")
                nc.tensor.matmul(ps[:], row_sb[:], one1[:], start=True, stop=True)
                fws = x_pool.tile(
                    [P, NCLS], f16, name=f"fws{ti}_{sub}", tag=f"fws{ti}{sub}"
                )
                nc.vector.tensor_scalar_mul(fws[:], fwt[:, sub, :], ps[:])
                return fws

            # a finishes mid-stream: its cos-side output matmuls overlap
            # the b stream; b's sin side forms the (short) tail.
            for sub in range(NSUB):
                fws = make_fws(0, sub)
                nc.tensor.matmul(
                    out_ps[:], fws[:], coss[sub][:], start=(sub == 0), stop=False
                )
            for (c0, c1) in B_PIECES:
                chunk_mms(1, b_sb, c0, c1)
            for sub in range(NSUB):
                fws = make_fws(1, sub)
                nc.tensor.matmul(
                    out_ps[:], fws[:], sins[sub][:],
                    start=False, stop=(sub == NSUB - 1),
                )

            # ---------- output store ----------
            out_sb = const_pool.tile([NCLS, B], f32, name="out_sb")
            nc.scalar.copy(out_sb[:, 0:H], out_ps[:, 0:H])
            nc.vector.tensor_copy(out_sb[:, H:B], out_ps[:, H:B])
            PH = NCLS // 2
            nc.sync.dma_start(out=out_cb[0:PH, :], in_=out_sb[0:PH, :])
            nc.scalar.dma_start(out=out_cb[PH:NCLS, :], in_=out_sb[PH:NCLS, :])

    nc.compile()
    return nc


def _get_nc():
    if "nc" not in _STATE:
        _STATE["nc"] = _build()
    return _STATE["nc"]


def _pack_ab(t2, sl):
    """[D, DW] f32 slice -> [P, NCH*DW] fp16, chunk-major free dim."""
    s = t2[:, sl].reshape(NCH, P, DW).transpose(1, 0, 2).reshape(P, NCH * DW)
    return np.ascontiguousarray(s.astype(np.float16))


def _prep_in_maps(x, a, b, fc_w):
    xf = np.asarray(x, dtype=np.float32).reshape(B, D)
    xt = xf.T.astype(np.float16)  # [D, B]
    a2 = np.asarray(a, dtype=np.float32).reshape(D, D)
    b2 = np.asarray(b, dtype=np.float32).reshape(D, D)
    fw = np.asarray(fc_w, dtype=np.float32)
    in_maps = []
    for m in range(NCORES):
        sl = slice(m * DW, (m + 1) * DW)
        # combo [P, NSUB, B+NCLS]: xt part + fwt part per sub
        xs = xt[sl, :].reshape(NSUB, P, B)
        fs = fw[:, sl].T.reshape(NSUB, P, NCLS).astype(np.float16)
        cmb = np.concatenate([xs, fs], axis=2)          # [NSUB, P, B+NCLS]
        cmb = np.ascontiguousarray(
            cmb.transpose(1, 0, 2).reshape(P, NSUB * (B + NCLS))
        )
        in_maps.append(
            {
                "a_s": _pack_ab(a2, sl),
                "b_s": _pack_ab(b2, sl),
                "cmb_s": cmb,
            }
        )
    return in_maps


def _run(inputs, trace=False, trace_kwargs=None):
    """Run the device kernel; returns (final_output, BassKernelResults)."""
    from concourse.bass_utils import run_bass_kernel_spmd

    x = inputs["x"]
    a = inputs["a"]
    b = inputs["b"]
    w = np.asarray(inputs["w"], dtype=np.float64)
    n_param = np.asarray(inputs["n_param"], dtype=np.float64)
    fc_w = np.asarray(inputs["fc_w"], dtype=np.float32)
    fc_b = np.asarray(inputs["fc_b"], dtype=np.float32)

    nc = _get_nc()
    in_maps = _prep_in_maps(x, a, b, fc_w)
    res = run_bass_kernel_spmd(
        nc,
        in_maps,
        list(range(NCORES)),
        trace=trace,
        **(trace_kwargs or {}),
    )

    acc = np.zeros((NCLS, B), dtype=np.float32)
    for r in res.results:
        acc += r["out_cb"]
    part1 = float(np.sum(w[1:] * n_param[1:] + w[:-1] * n_param[:-1]))
    final = acc.T + np.float32(part1) * fc_w.sum(axis=1)[None, :] + fc_b[None, :]
    return np.ascontiguousarray(final.astype(np.float32)), res


def kernel(**inputs) -> np.ndarray:
    out, _ = _run(inputs, trace=False)
    return out


# revision 17
# speedup vs baseline: 1.2078x; 1.2078x over previous
"""Trainium2 Bass kernel for nn_CustomCIFAR10Model.

Math (reference):
    xf = x.reshape(B, D)
    part2[b,d] = cos(xf[b,d]) * Sa[d] + sin(xf[b,d]) * Sb[d]
        where Sa[d] = sum_i a[i,d,0], Sb[d] = sum_i b[i,d,0]
    part1 = sum(w[1:]*n[1:] + w[:-1]*n[:-1])            (scalar)
    out = (part1 + part2) @ fc_w.T + fc_b               [B, NCLS]

Memory-bound: the only heavy work is streaming a and b once to
column-sum them. Sharding: d-columns split across 8 cores (384 each);
every core reduces its a/b slice, scales its fc_w columns by the sums,
and contracts against cos/sin(x) to a partial [NCLS, B] output; the
host sums the 8 partials and adds part1/bias.

Schedule (evolved over several traced iterations):
 - a/b/x/fc_w cast to fp16 on the host: the stream halves to ~5.2 MB
   per core (adds ~4e-4 relative error, gate is 2e-2).
 - ALL load DMAs ride the single sync HWDGE ring, issued back-to-back
   at kernel start.  The stream is HBM-bound (~390-420 GB/s either
   way), and a single FIFO ring makes the arrival order deterministic:
   PE matmuls are emitted in exactly ring order, so the in-order PE
   never stalls on a not-yet-landed piece while a later piece sits
   ready (cost 2-4 us in every dual-ring variant).  It also keeps DMA
   issue off the ACT engine: a 9th+ DMA's issue blocks its engine
   until a completion-sem lane recycles, which on ACT pushed the Sin
   table load + all Sins back by ~4 us.
 - Piece order: combo (x+fc_w) first -> trig pipeline (DVE+ACT) runs
   11-16 us; a pieces small->big (PE starts ~10.7, stays fed); b big->
   small (the last piece is 2 chunks, so the post-stream tail starts
   with only ~0.3 us of reduction left).
 - Elementwise trig on DVE in fp16 (2x mode) + Sin on ACT.  The magic
   round constant stays 1.5*2^23: DVE computes INTERNALLY in fp32, so
   the fused (t+M)-M tensor_scalar rounds exactly and only the small
   integer result lands in fp16.  GpSimd runs nothing (its tensor ops
   measure ~20x slower than DVE, and its SWDGE DMA path took 10 us for
   a 0.15 MB tensor).
 - 14 dummy matmuls bridge the PE from preamble end to first data so
   the HAM clock-gate reaches 2.4 GHz (cold 1.2 GHz doubles the 7.7 us
   reduction; fewer warm-ups measurably failed to flip it).
 - Output store: PSUM->SBUF copy split by columns across ACT/DVE, then
   two partition-contiguous DMAs on the two HWDGE rings.

HW Sin only accepts [-pi, pi]: range-reduce t = x/(2pi), r = t - round(t)
via the magic trick, then Sin(2pi*r); cos shifts t by +1/4 first.
"""

import numpy as np

B = 512
D = 3072
NCLS = 100
P = 128
NCORES = 8
DW = D // NCORES          # 384 columns per core
NSUB = DW // P            # 3 d-subtiles of 128
NCH = D // P              # 24 row-chunks of the a/b slice

# Chunk ownership: a_lo/b_hi load on the sync ring, a_hi/b_lo on the
# scalar ring (each tile written by exactly one ring -- cross-ring
# writes to one tile serialize on a hazard sem).  Dual rings beat one:
# ~420 GB/s combined vs ~330-390 single (each ring's inter-DMA bubbles
# are hidden by the other).  DMA *emission* order interleaves the rings
# so the scalar ring's four DMAs take early completion-sem lanes and
# its engine (ACT) never stalls waiting for a lane to recycle.
ALO = 10   # a chunks 0..ALO-1 on sync
BLO = 14   # b chunks 0..BLO-1 on scalar

_STATE = {}


def _build():
    """Build + bacc-compile the SPMD Bass program (once per process)."""
    import concourse.bacc as bacc
    import concourse.mybir as mybir
    import concourse.tile as tile

    f32 = mybir.dt.float32
    f16 = mybir.dt.float16
    nc = bacc.Bacc(
        "TRN2", target_bir_lowering=False, debug=False, num_devices=NCORES
    )

    a_s = nc.dram_tensor("a_s", [P, NCH * DW], f16, kind="ExternalInput")
    b_s = nc.dram_tensor("b_s", [P, NCH * DW], f16, kind="ExternalInput")
    # combo: per partition p, per sub s: [ x (B) | fwt (NCLS) ]
    cmb_s = nc.dram_tensor("cmb_s", [P, NSUB * (B + NCLS)], f16, kind="ExternalInput")
    out_cb = nc.dram_tensor("out_cb", [NCLS, B], f32, kind="ExternalOutput")

    INV2PI = float(1.0 / (2.0 * np.pi))
    TWO_PI = float(2.0 * np.pi)
    MAGIC = float(1.5 * 2.0**23)
    add_op = mybir.AluOpType.add
    sub_op = mybir.AluOpType.subtract
    Sin = mybir.ActivationFunctionType.Sin

    with tile.TileContext(nc) as tc:
        with (
            tc.tile_pool(name="consts", bufs=1) as const_pool,
            tc.tile_pool(name="xwork", bufs=1) as x_pool,
            tc.tile_pool(name="ps", bufs=2, space="PSUM") as psum_pool,
            tc.tile_pool(name="psrow", bufs=1, space="PSUM") as psum_row_pool,
            tc.tile_pool(name="psout", bufs=1, space="PSUM") as psum_out_pool,
            tc.tile_pool(name="pswarm", bufs=1, space="PSUM") as psum_warm_pool,
        ):
            ones_h = const_pool.tile([P, 1], f16, name="ones_h")
            nc.vector.memset(ones_h[:], 1.0)
            one1 = const_pool.tile([1, 1], f32, name="one1")
            nc.vector.memset(one1[:], 1.0)
            zero = const_pool.tile([P, 1], f32, name="zerob")
            nc.vector.memset(zero[:], 0.0)
            zero_h = const_pool.tile([P, 1], f16, name="zero_h")
            nc.vector.memset(zero_h[:], 0.0)
            warmz = const_pool.tile([P, DW], f16, name="warmz")
            nc.vector.memset(warmz[:], 0.0)

            # ---------- load DMAs ----------
            a_lo = x_pool.tile([P, ALO, DW], f16, name="a_lo")
            a_hi = x_pool.tile([P, NCH - ALO, DW], f16, name="a_hi")
            b_lo = x_pool.tile([P, BLO, DW], f16, name="b_lo")
            b_hi = x_pool.tile([P, NCH - BLO, DW], f16, name="b_hi")
            cmb = x_pool.tile([P, NSUB, B + NCLS], f16, name="cmb")

            def ld(eng, dst, src, base, c0, c1):
                eng.dma_start(
                    out=dst[:, c0:c1, :],
                    in_=src[:, (base + c0) * DW : (base + c1) * DW],
                )

            # Emission (= sem-lane) order interleaves rings: scalar's five
            # DMAs land in lanes 1,2,4,6,7; only sync DMAs (idle engine)
            # take recycled lanes.
            nc.sync.dma_start(
                out=cmb[:], in_=cmb_s[:].rearrange("p (s c) -> p s c", s=NSUB)
            )
            ld(nc.scalar, a_hi, a_s, ALO, 0, 7)
            ld(nc.scalar, a_hi, a_s, ALO, 7, 14)
            ld(nc.sync, a_lo, a_s, 0, 0, 5)
            ld(nc.scalar, b_lo, b_s, 0, 0, 8)
            ld(nc.sync, a_lo, a_s, 0, 5, 10)
            ld(nc.scalar, b_lo, b_s, 0, 8, 12)
            ld(nc.sync, b_hi, b_s, BLO, 0, 5)
            ld(nc.scalar, b_lo, b_s, 0, 12, 14)
            ld(nc.sync, b_hi, b_s, BLO, 5, 8)
            ld(nc.sync, b_hi, b_s, BLO, 8, 10)

            # Dummy Sin so the Sin table set loads once at kernel start.
            warm = const_pool.tile([P, 1], f32, name="warm")
            nc.scalar.activation(warm[:], zero[:], Sin, bias=zero[:])

            # PE warm-up bridge: preamble end -> first data arrival.
            warm_ps = psum_warm_pool.tile([1, DW], f32, name="warm_ps")
            for _ in range(14):
                nc.tensor.matmul(
                    warm_ps[:], ones_h[:], warmz[:], start=True, stop=True
                )

            # ---------- column-sum reduction (PE) ----------
            rows = []
            for ti in range(2):
                psr = psum_row_pool.tile([1, DW], f32, name=f"psr{ti}", tag=f"psr{ti}")
                rows.append(psr)
            emitted = [0, 0]

            def chunk_mms(ti, src, c0, c1):
                for c in range(c0, c1):
                    nc.tensor.matmul(
                        rows[ti][:],
                        ones_h[:],
                        src[:, c, :],
                        start=(emitted[ti] == 0),
                        stop=(emitted[ti] == NCH - 1),
                    )
                    emitted[ti] += 1

            # a chunks in measured ring-arrival order
            chunk_mms(0, a_hi, 0, 7)
            chunk_mms(0, a_lo, 0, 5)
            chunk_mms(0, a_hi, 7, 14)
            chunk_mms(0, a_lo, 5, 10)

            # ---------- trig on x: fp16 on DVE + ACT, cos side first ----
            xt = cmb[:, :, 0:B]
            fwt = cmb[:, :, B : B + NCLS]
            sins = []
            coss = []
            ts_ts = []
            for sub in range(NSUB):
                xts = xt[:, sub, :]
                ts_t = x_pool.tile([P, B], f16, name=f"ts{sub}")
                nc.vector.tensor_scalar_mul(ts_t[:], xts, INV2PI)
                ts_ts.append(ts_t)
                tc_t = x_pool.tile([P, B], f16, name=f"tc{sub}")
                nc.vector.tensor_scalar_add(tc_t[:], ts_t[:], 0.25)
                kc_t = x_pool.tile([P, B], f16, name=f"kc{sub}")
                nc.vector.tensor_scalar(kc_t[:], tc_t[:], MAGIC, MAGIC, add_op, sub_op)
                rc_t = x_pool.tile([P, B], f16, name=f"rc{sub}")
                nc.vector.tensor_sub(rc_t[:], tc_t[:], kc_t[:])
                cosv = x_pool.tile([P, B], f16, name=f"cos{sub}")
                nc.scalar.activation(cosv[:], rc_t[:], Sin, bias=zero_h[:], scale=TWO_PI)
                coss.append(cosv)
            for sub in range(NSUB):
                ks_t = x_pool.tile([P, B], f16, name=f"ks{sub}")
                nc.vector.tensor_scalar(ks_t[:], ts_ts[sub][:], MAGIC, MAGIC, add_op, sub_op)
                rs_t = x_pool.tile([P, B], f16, name=f"rs{sub}")
                nc.vector.tensor_sub(rs_t[:], ts_ts[sub][:], ks_t[:])
                sinv = x_pool.tile([P, B], f16, name=f"sin{sub}")
                nc.scalar.activation(sinv[:], rs_t[:], Sin, bias=zero_h[:], scale=TWO_PI)
                sins.append(sinv)

            out_ps = psum_out_pool.tile([NCLS, B], f32, name="out_ps")
            H = B // 2

            def make_fws(ti, sub):
                """Transpose row ti/sub to per-partition cols (via PE),
                scale the SMALL fwt tile by it (fwt[d,c]*S[d])."""
                row_sb = const_pool.tile(
                    [1, P], f32, name=f"row{ti}_{sub}", tag=f"row{ti}_{sub}"
                )
                nc.scalar.copy(row_sb[:], rows[ti][0:1, sub * P : (sub + 1) * P])
                ps = psum_pool.tile([P, 1], f32, name=f"ps{ti}_{sub}", tag="ps")
                nc.tensor.matmul(ps[:], row_sb[:], one1[:], start=True, stop=True)
                fws = x_pool.tile(
                    [P, NCLS], f16, name=f"fws{ti}_{sub}", tag=f"fws{ti}{sub}"
                )
                nc.vector.tensor_scalar_mul(fws[:], fwt[:, sub, :], ps[:])
                return fws

            # a finishes mid-stream: its cos-side output matmuls overlap
            # the b stream; b's sin side forms the (short) tail.
            for sub in range(NSUB):
                fws = make_fws(0, sub)
                nc.tensor.matmul(
                    out_ps[:], fws[:], coss[sub][:], start=(sub == 0), stop=False
                )
            chunk_mms(1, b_lo, 0, 8)
            chunk_mms(1, b_hi, 0, 5)
            chunk_mms(1, b_lo, 8, 12)
            chunk_mms(1, b_hi, 5, 8)
            chunk_mms(1, b_lo, 12, 14)
            chunk_mms(1, b_hi, 8, 10)
            for sub in range(NSUB):
                fws = make_fws(1, sub)
                nc.tensor.matmul(
                    out_ps[:], fws[:], sins[sub][:],
                    start=False, stop=(sub == NSUB - 1),
                )

            # ---------- output store ----------
            out_sb = const_pool.tile([NCLS, B], f32, name="out_sb")
            nc.scalar.copy(out_sb[:, 0:H], out_ps[:, 0:H])
            nc.vector.tensor_copy(out_sb[:, H:B], out_ps[:, H:B])
            PH = NCLS // 2
            nc.sync.dma_start(out=out_cb[0:PH, :], in_=out_sb[0:PH, :])
            nc.scalar.dma_start(out=out_cb[PH:NCLS, :], in_=out_sb[PH:NCLS, :])

    nc.compile()
    return nc


def _get_nc():
    if "nc" not in _STATE:
        _STATE["nc"] = _build()
    return _STATE["nc"]


def _pack_ab(t2, sl):
    """[D, DW] f32 slice -> [P, NCH*DW] fp16, chunk-major free dim."""
    s = t2[:, sl].reshape(NCH, P, DW).transpose(1, 0, 2).reshape(P, NCH * DW)
    return np.ascontiguousarray(s.astype(np.float16))


def _prep_in_maps(x, a, b, fc_w):
    xf = np.asarray(x, dtype=np.float32).reshape(B, D)
    xt = xf.T.astype(np.float16)  # [D, B]
    a2 = np.asarray(a, dtype=np.float32).reshape(D, D)
    b2 = np.asarray(b, dtype=np.float32).reshape(D, D)
    fw = np.asarray(fc_w, dtype=np.float32)
    in_maps = []
    for m in range(NCORES):
        sl = slice(m * DW, (m + 1) * DW)
        # combo [P, NSUB, B+NCLS]: xt part + fwt part per sub
        xs = xt[sl, :].reshape(NSUB, P, B)
        fs = fw[:, sl].T.reshape(NSUB, P, NCLS).astype(np.float16)
        cmb = np.concatenate([xs, fs], axis=2)          # [NSUB, P, B+NCLS]
        cmb = np.ascontiguousarray(
            cmb.transpose(1, 0, 2).reshape(P, NSUB * (B + NCLS))
        )
        in_maps.append(
            {
                "a_s": _pack_ab(a2, sl),
                "b_s": _pack_ab(b2, sl),
                "cmb_s": cmb,
            }
        )
    return in_maps


def _run(inputs, trace=False, trace_kwargs=None):
    """Run the device kernel; returns (final_output, BassKernelResults)."""
    from concourse.bass_utils import run_bass_kernel_spmd

    x = inputs["x"]
    a = inputs["a"]
    b = inputs["b"]
    w = np.asarray(inputs["w"], dtype=np.float64)
    n_param = np.asarray(inputs["n_param"], dtype=np.float64)
    fc_w = np.asarray(inputs["fc_w"], dtype=np.float32)
    fc_b = np.asarray(inputs["fc_b"], dtype=np.float32)

    nc = _get_nc()
    in_maps = _prep_in_maps(x, a, b, fc_w)
    res = run_bass_kernel_spmd(
        nc,
        in_maps,
        list(range(NCORES)),
        trace=trace,
        **(trace_kwargs or {}),
    )

    acc = np.zeros((NCLS, B), dtype=np.float32)
    for r in res.results:
        acc += r["out_cb"]
    part1 = float(np.sum(w[1:] * n_param[1:] + w[:-1] * n_param[:-1]))
    final = acc.T + np.float32(part1) * fc_w.sum(axis=1)[None, :] + fc_b[None, :]
    return np.ascontiguousarray(final.astype(np.float32)), res


def kernel(**inputs) -> np.ndarray:
    out, _ = _run(inputs, trace=False)
    return out
